# revision 1
# baseline (speedup 1.0000x reference)
"""Trainium2 Bass kernel for nn_IngredientScannerLoss.

Per row (12 coords = 6 (x,y) pairs):
    delta = output - target
    dist_j = sqrt(dx_j^2 + dy_j^2)
    n_j    = (s0_j*dx_j > 0) + (s1_j*dy_j > 0)   (sign-gated count, 0/1/2)
    f(x)   = ((x+1)^1.2 - 1)*2
    t_j    = [dist, f(dist), f(f(dist))][n_j]
    loss   = sum_j t_j

Data-parallel over 8 NeuronCores: rows split 8 x 500_000, each shard
zero-padded to 501_760 = 128*3920 rows so tiles are [128, RT*12].

Engine split per tile:
    GPSIMD: delta = a - b                       (tensor_tensor subtract)
    DVE:    s = dx^2+dy^2 (custom op), n (custom op x6 pair columns),
            d1/d2 affines, predicated selects, row-sum reduce
    ACT:    ln/exp chains (single natural_log_exp table set; sqrt is done
            as exp(0.5*ln s) to avoid table switches)
"""

import numpy as np

import concourse.bacc as bacc
import concourse.bass as bass
import concourse.mybir as mybir
import concourse.tile as tile
from concourse import dve_ops
from concourse.bass_utils import run_bass_kernel_spmd
from concourse.dve_ops import DveOp
from concourse.dve_spec import Spec, Src0, Src1, C0, C1, Zero, _has_src1, lower, sq
from concourse.dve_uop import DveOpSpec

P = 128
COLS = 12
NPAIR = 6
B = 4_000_000
N_CORES = 8
ROWS_VALID = B // N_CORES          # 500_000
RT = 392                           # rows per partition per tile
NT = 10                            # tiles per core
ROWS_PC = P * RT * NT              # 501_760 padded rows per core
LN2 = 0.6931471805599453

# per-coordinate condition signs (see reference _SIGNS)
SIGNS = [1.0, 1.0, 1.0, -1.0, -1.0, -1.0, -1.0, 1.0, 0.0, 1.0, 0.0, -1.0]

F32 = mybir.dt.float32
AF = mybir.ActivationFunctionType
ALU = mybir.AluOpType

# how many pair columns can ever hit n == 2 (pairs 4,5 have s0 == 0 -> n <= 1,
# so the second transform is only needed for pair columns 0..3)
NPAIR2 = 4

# ---------------------------------------------------------------- custom ops


def _register_op(name: str, spec: Spec, subdim: bool = False) -> DveOp:
    for op in dve_ops.OPS:
        if op.name == name:
            return op
    if name not in dve_ops._SUB_OPCODE_FOR_NAME:
        row = max(dve_ops._SUB_OPCODE_FOR_NAME.values()) + 1
        assert row < 0x20, "custom DVE opcode rows exhausted"
        dve_ops._SUB_OPCODE_FOR_NAME[name] = row
    shas = {}
    for ver in ("v3", "v4"):
        try:
            shas[ver] = DveOpSpec(
                name=name,
                opcode=dve_ops.get_dve_sub_opcode(name),
                uops=lower(spec, ver=ver),
                rd1_en=_has_src1(spec),
            ).sha(ver)
        except Exception:
            pass
    op = DveOp(name, spec, subdim, shas)
    dve_ops.OPS.append(op)
    dve_ops.CUSTOM_DVE_SPECS[name] = spec
    return op


# s = in0^2 + in1^2  (in0/in1 = even/odd delta columns)
PAIRDIST = _register_op(
    "ANT_PAIRDIST",
    Spec(
        body=sq(Src0) + sq(Src1),
        reference=lambda in0, in1, s0, s1, imm2: (
            in0.astype(np.float32) ** 2 + in1.astype(np.float32) ** 2
        ),
    ),
)

# n = (in0*s0 > 0) + (in1*s1 > 0)
CGATE = _register_op(
    "ANT_CGATE",
    Spec(
        body=(Src0 * C0 > Zero) + (Src1 * C1 > Zero),
        reference=lambda in0, in1, s0, s1, imm2: (
            ((in0.astype(np.float32) * s0) > 0).astype(np.float32)
            + ((in1.astype(np.float32) * s1) > 0).astype(np.float32)
        ),
    ),
)


# ---------------------------------------------------------------- act tables
# The stock table-load pass resolves Exp -> exp_and_others and
# Ln -> natural_log, reloading ACT tables on every Ln<->Exp switch
# (~2.7us each, ~100us/core total). Restrict ln/exp membership to sets
# that hold BOTH so every activation resolves to
# natural_log_exp_and_others and the load hoists to one per kernel.
# Dict order (and thus act_func_set_id indices) is preserved.

_GAT_REAL = None


def _gat_lnexp(arch):
    global _GAT_REAL
    from concourse.hw_specs import get_activation_tables

    if _GAT_REAL is None:
        _GAT_REAL = get_activation_tables
    tabs = _GAT_REAL(arch)
    out = {}
    for name, funcs in tabs.items():
        fs = set(funcs)
        if not (AF.Ln in fs and AF.Exp in fs):
            fs.discard(AF.Ln)
            fs.discard(AF.Exp)
        out[name] = fs
    return out


def _patch_act_tables():
    if bacc.get_activation_tables is not _gat_lnexp:
        global _GAT_REAL
        _GAT_REAL = bacc.get_activation_tables
        bacc.get_activation_tables = _gat_lnexp


# ---------------------------------------------------------------- bass build


def build_nc(rt: int = RT, nt: int = NT):
    """Build the single-core SPMD program for [P*rt*nt, 12] inputs."""
    _patch_act_tables()
    rows = P * rt * nt
    nc = bacc.Bacc("TRN2", debug=False, target_bir_lowering=False,
                   num_devices=N_CORES)
    # activation biases need registered const APs (only 0.0/1.0 ship)
    for cv in (-1.0, LN2):
        if (F32, cv) not in nc.const_aps.aps:
            ct = nc.alloc_sbuf_tensor(f"const-f32-{cv}", [P, 1], F32)
            nc.gpsimd.memset(ct.ap(), cv)
            nc.const_aps.aps[(F32, cv)] = ct.ap()
    nc.all_engine_barrier()
    a = nc.dram_tensor("output", [rows, COLS], F32, kind="ExternalInput").ap()
    b = nc.dram_tensor("target", [rows, COLS], F32, kind="ExternalInput").ap()
    o = nc.dram_tensor("loss", [rows], F32, kind="ExternalOutput").ap()

    a3 = a.rearrange("(n p r) m -> n p (r m)", p=P, r=rt)
    b3 = b.rearrange("(n p r) m -> n p (r m)", p=P, r=rt)
    o3 = o.rearrange("(n p r) -> n p r", p=P, r=rt)

    I32 = mybir.dt.int32
    with tile.TileContext(nc) as tc:
        with tc.tile_pool(name="sb", bufs=2) as pool:
            for i in range(nt):
                w4 = rt * NPAIR2
                w6 = rt * NPAIR
                ta = pool.tile([P, rt * COLS], F32, tag="ta")
                nc.sync.dma_start(out=ta[:], in_=a3[i])
                tb = pool.tile([P, rt * COLS], F32, tag="tb")
                nc.sync.dma_start(out=tb[:], in_=b3[i])

                # delta split into even/odd coordinate blocks, pair-major:
                # delta[:, comp*6rt + j*rt + r] = a[12r+2j+comp]-b[...]
                delta = pool.tile([P, rt * COLS], F32, tag="delta")
                d4 = delta[:].rearrange("p (two j r) -> p two j r",
                                        two=2, j=NPAIR)
                a4 = ta[:].rearrange("p (r j two) -> p two j r",
                                     two=2, j=NPAIR)
                b4 = tb[:].rearrange("p (r j two) -> p two j r",
                                     two=2, j=NPAIR)
                nc.gpsimd.tensor_tensor(d4, a4, b4, ALU.subtract)
                dE = delta[:, 0:w6]
                dO = delta[:, w6:2 * w6]

                # s = dx^2 + dy^2, [P, 6*rt] pair-major (contiguous ins)
                s = pool.tile([P, w6], F32, tag="s")
                nc.vector._custom_dve(PAIRDIST, out=s[:], in0=dE, in1=dO)

                # n gates, [P, 6*rt] pair-major. Pairs 4,5 have s0==0 so
                # n = (s1*dy > 0): single-src tensor_scalar (2x mode).
                n = pool.tile([P, w6], F32, tag="n")
                for j in range(NPAIR):
                    xs = slice(j * rt, (j + 1) * rt)
                    if SIGNS[2 * j] != 0.0:
                        nc.vector._custom_dve(
                            CGATE,
                            out=n[:, xs],
                            in0=delta[:, j * rt:(j + 1) * rt],
                            in1=delta[:, w6 + j * rt:w6 + (j + 1) * rt],
                            s0=SIGNS[2 * j],
                            s1=SIGNS[2 * j + 1],
                        )
                    else:
                        op = ALU.is_gt if SIGNS[2 * j + 1] > 0 else ALU.is_lt
                        nc.vector.tensor_scalar(
                            n[:, xs], delta[:, w6 + j * rt:w6 + (j + 1) * rt],
                            0.0, None, op)

                # ACT chain, one table set (ln+exp):
                #   lt  = ln(s)                  (in-place on s)
                #   res = exp(0.5*lt) = dist
                #   t   = ln(res + 1)
                #   W0  = exp(1.2*t + ln2) = 2u  (in-place on t)
                #   t2  = ln(W0 - 1) = ln(2u-1)
                #   W1  = exp(1.2*t2 + ln2) = 2v (in-place on t2)
                nc.scalar.activation(s[:], s[:], AF.Ln)
                res = pool.tile([P, w6], F32, tag="res")
                nc.scalar.activation(res[:], s[:], AF.Exp, scale=0.5)
                t = pool.tile([P, w6], F32, tag="t")
                nc.scalar.activation(t[:], res[:], AF.Ln, bias=1.0)
                nc.scalar.activation(t[:], t[:], AF.Exp, scale=1.2, bias=LN2)
                t2 = pool.tile([P, w4], F32, tag="t2")
                nc.scalar.activation(t2[:], t[:, 0:w4], AF.Ln, bias=-1.0)
                nc.scalar.activation(t2[:], t2[:], AF.Exp, scale=1.2, bias=LN2)

                # d1 = W0 - 2 = 2u - 2 (in-place), d2 = W1 - 2 (in-place)
                nc.vector.tensor_scalar(t[:], t[:], 2.0, None, ALU.subtract)
                nc.vector.tensor_scalar(t2[:], t2[:], 2.0, None, ALU.subtract)

                # res (= dist) overwritten by d1 where n>=1, d2 where n>=2.
                # CopyPredicated wants an integer mask; fp32 {0.,1.,2.}
                # bitcast to int32 is nonzero exactly where the float is.
                nc.vector.copy_predicated(res[:], n[:].bitcast(I32), t[:])
                # m2 = relu(n-1), in-place on n prefix (after cp1 read n)
                nc.vector.tensor_scalar(n[:, 0:w4], n[:, 0:w4], 1.0, 0.0,
                                        ALU.subtract, ALU.max)
                nc.vector.copy_predicated(res[:, 0:w4],
                                          n[:, 0:w4].bitcast(I32), t2[:])

                # row sums via contiguous add tree (pair order irrelevant)
                w3 = rt * 3
                nc.vector.tensor_tensor(res[:, 0:w3], res[:, 0:w3],
                                        res[:, w3:2 * w3], ALU.add)
                nc.vector.tensor_tensor(res[:, 0:rt], res[:, 0:rt],
                                        res[:, rt:2 * rt], ALU.add)
                ot = pool.tile([P, rt], F32, tag="ot")
                nc.vector.tensor_tensor(ot[:], res[:, 0:rt],
                                        res[:, 2 * rt:w3], ALU.add)
                nc.sync.dma_start(out=o3[i], in_=ot[:])
    nc.compile()
    return nc


_NC_CACHE: dict = {}


def _get_nc(rt: int = RT, nt: int = NT):
    key = (rt, nt)
    if key not in _NC_CACHE:
        _NC_CACHE[key] = build_nc(rt, nt)
    return _NC_CACHE[key]


# ---------------------------------------------------------------- entrypoint


def kernel(output, target):
    a = np.asarray(output, dtype=np.float32)
    b = np.asarray(target, dtype=np.float32)
    assert a.shape == (B, COLS) and b.shape == (B, COLS)

    a_sh = np.zeros((N_CORES, ROWS_PC, COLS), dtype=np.float32)
    b_sh = np.zeros((N_CORES, ROWS_PC, COLS), dtype=np.float32)
    a_sh[:, :ROWS_VALID, :] = a.reshape(N_CORES, ROWS_VALID, COLS)
    b_sh[:, :ROWS_VALID, :] = b.reshape(N_CORES, ROWS_VALID, COLS)

    nc = _get_nc()
    in_maps = [
        {"output": a_sh[c], "target": b_sh[c]} for c in range(N_CORES)
    ]
    r = run_bass_kernel_spmd(nc, in_maps, list(range(N_CORES)))
    out = np.empty((N_CORES, ROWS_VALID), dtype=np.float32)
    for c in range(N_CORES):
        out[c] = r.results[c]["loss"][:ROWS_VALID]
    return out.reshape(B)



# revision 4
# speedup vs baseline: 1.1922x; 1.1922x over previous
"""Trainium2 Bass kernel for nn_IngredientScannerLoss.

Per row (12 coords = 6 (x,y) pairs):
    delta = output - target
    dist_j = sqrt(dx_j^2 + dy_j^2)
    n_j    = (s0_j*dx_j > 0) + (s1_j*dy_j > 0)   (sign-gated count, 0/1/2)
    f(x)   = ((x+1)^1.2 - 1)*2
    t_j    = [dist, f(dist), f(f(dist))][n_j]
    loss   = sum_j t_j

Data-parallel over 8 NeuronCores: rows split 8 x 500_000, each shard
zero-padded to 501_760 = 128*RT*NT rows.

v2 design (vs v1 at 343us):
  * Host stores shards component-major: per tile [P, 12*RT] where the
    first 6 RT-blocks are x-components and the last 6 are y-components,
    pairs permuted to O2 = [2,3,0,1,5,4] so that
      - x-block sign runs:  [-,-,+,+,0,0] -> 2 subtract instructions
      - y-block sign runs:  [-,+,+,-,-,+] -> 4 subtract instructions
      - pairs that can reach n=2 occupy [0,4RT) contiguously
    Pool computes SIGNED delta d' = sign*(a-b) by swapping operands per
    run (contiguous access; v1's strided subtract ran 5x below roofline).
    Squaring kills the sign for dist; gates become (d' > 0).
  * All selection logic is plain bf16 tensor_tensor/tensor_scalar (DVE
    2x_1p mode) -- no multi-uop custom DVE ops, no copy_predicated.
    Select-by-max: res = max(dist, n*d1, m2*d2), valid because
    f(x) >= 2.4x >= 0 on x>=0 so d2 >= 2*d1 >= ... >= dist >= n-scaled
    lower candidates.
  * ACT chain keeps log-space values f32 (bf16 there fails tolerance),
    value-space outputs bf16. Single ln+exp table set (patched tables).
  Expected engine busy: DMA ~141us (floor), DVE ~145, ACT ~122, Pool ~100.
"""

import numpy as np

import concourse.bacc as bacc
import concourse.bass as bass
import concourse.mybir as mybir
import concourse.tile as tile
from concourse.bass_utils import run_bass_kernel_spmd

P = 128
COLS = 12
NPAIR = 6
B = 4_000_000
N_CORES = 8
ROWS_VALID = B // N_CORES          # 500_000
RT = 392                           # rows per partition per tile
NT = 10                            # tiles per core
ROWS_PC = P * RT * NT              # 501_760 padded rows per core
LN2 = 0.6931471805599453

# original per-coordinate condition signs (reference _SIGNS)
SIGNS = [1.0, 1.0, 1.0, -1.0, -1.0, -1.0, -1.0, 1.0, 0.0, 1.0, 0.0, -1.0]

# pair permutation: block position -> original pair index
PAIR_ORDER = [2, 3, 0, 1, 5, 4]
# component order for the host layout: x-block then y-block
COMP_PERM = [2 * j for j in PAIR_ORDER] + [2 * j + 1 for j in PAIR_ORDER]
# number of leading pair positions that can reach n == 2
NPAIR2 = 4

F32 = mybir.dt.float32
BF16 = mybir.dt.bfloat16
AF = mybir.ActivationFunctionType
ALU = mybir.AluOpType

# subtract runs (start_pair_block, end_pair_block, swap) over the 12 blocks:
# block b covers cols [b*RT, (b+1)*RT); sign of block = SIGNS[COMP_PERM[b]];
# swap=True -> compute b-a (negative sign), sign-0 blocks merged arbitrarily.
SUB_RUNS = [
    (0, 2, True),    # x pairs 2,3: sign -1
    (2, 6, False),   # x pairs 0,1 (+1) and 5,4 (0)
    (6, 7, True),    # y pair 2: -1
    (7, 9, False),   # y pairs 3,0: +1
    (9, 11, True),   # y pairs 1,5: -1
    (11, 12, False), # y pair 4: +1
]

# ---------------------------------------------------------------- act tables
# The stock table-load pass resolves Exp -> exp_and_others and
# Ln -> natural_log, reloading ACT tables on every Ln<->Exp switch
# (~1.3us each). Restrict ln/exp membership to sets that hold BOTH so
# every activation resolves to natural_log_exp_and_others and the load
# hoists to one per kernel. Dict order (act_func_set_id) is preserved.

_GAT_REAL = None


def _gat_lnexp(arch):
    global _GAT_REAL
    from concourse.hw_specs import get_activation_tables

    if _GAT_REAL is None:
        _GAT_REAL = get_activation_tables
    tabs = _GAT_REAL(arch)
    out = {}
    for name, funcs in tabs.items():
        fs = set(funcs)
        if not (AF.Ln in fs and AF.Exp in fs):
            fs.discard(AF.Ln)
            fs.discard(AF.Exp)
        out[name] = fs
    return out


def _patch_act_tables():
    if bacc.get_activation_tables is not _gat_lnexp:
        global _GAT_REAL
        _GAT_REAL = bacc.get_activation_tables
        bacc.get_activation_tables = _gat_lnexp


# ---------------------------------------------------------------- bass build


def build_nc(rt: int = RT, nt: int = NT, bufs: int = 2):
    """Single-core SPMD program: inputs [nt, P, 12*rt] comp-major f32."""
    _patch_act_tables()
    nc = bacc.Bacc("TRN2", debug=False, target_bir_lowering=False,
                   num_devices=N_CORES)
    # activation biases need registered const APs (only 0.0/1.0 ship)
    for cv in (-1.0, LN2):
        if (F32, cv) not in nc.const_aps.aps:
            ct = nc.alloc_sbuf_tensor(f"const-f32-{cv}", [P, 1], F32)
            nc.gpsimd.memset(ct.ap(), cv)
            nc.const_aps.aps[(F32, cv)] = ct.ap()
    nc.all_engine_barrier()
    w12 = rt * COLS
    w6 = rt * NPAIR
    w4 = rt * NPAIR2
    w3 = rt * 3
    a = nc.dram_tensor("output", [nt, P, w12], F32, kind="ExternalInput").ap()
    b = nc.dram_tensor("target", [nt, P, w12], F32, kind="ExternalInput").ap()
    o = nc.dram_tensor("loss", [nt, P, rt], F32, kind="ExternalOutput").ap()

    with tile.TileContext(nc) as tc:
        with tc.tile_pool(name="sb", bufs=bufs) as pool:
            for i in range(nt):
                ta = pool.tile([P, w12], F32, tag="ta")
                nc.sync.dma_start(out=ta[:], in_=a[i])
                tb = pool.tile([P, w12], F32, tag="tb")
                nc.sync.dma_start(out=tb[:], in_=b[i])

                # signed delta d' on Pool, contiguous runs, f32 -> bf16
                delta = pool.tile([P, w12], BF16, tag="delta")
                for (lo, hi, swap) in SUB_RUNS:
                    xs = slice(lo * rt, hi * rt)
                    src0, src1 = (tb, ta) if swap else (ta, tb)
                    nc.gpsimd.tensor_tensor(
                        delta[:, xs], src0[:, xs], src1[:, xs], ALU.subtract)
                dx = delta[:, 0:w6]
                dy = delta[:, w6:w12]

                # s = dx^2 + dy^2 (bf16 tt, 2x mode)
                sqx = pool.tile([P, w6], BF16, tag="sqx")
                nc.vector.tensor_tensor(sqx[:], dx, dx, ALU.mult)
                s = pool.tile([P, w6], BF16, tag="s")
                nc.vector.tensor_tensor(s[:], dy, dy, ALU.mult)
                nc.vector.tensor_tensor(s[:], s[:], sqx[:], ALU.add)

                # gates: cy = (dy' > 0) everywhere; n = (dx' > 0) + cy on
                # the two-condition pairs [0,4RT). n aliases cy.
                n = pool.tile([P, w6], BF16, tag="n")
                nc.vector.tensor_scalar(n[:], dy, 0.0, None, ALU.is_gt)
                nc.vector.scalar_tensor_tensor(
                    n[:, 0:w4], delta[:, 0:w4], 0.0, n[:, 0:w4],
                    ALU.is_gt, ALU.add)
                # m2 = (n > 1), only needed on [0,4RT)
                m2 = pool.tile([P, w4], BF16, tag="m2")
                nc.vector.tensor_scalar(m2[:], n[:, 0:w4], 1.0, None,
                                        ALU.is_gt)

                # ACT chain, one ln+exp table set; log-space stays f32.
                # lt holds ls, then t, then t2 (all ACT-serial reuses):
                #   ls   = ln(s)                        f32
                #   dist = exp(0.5*ls)                  bf16
                #   t    = ln(dist + 1)                 f32
                #   W0   = exp(1.2*t + ln2) = d1 + 2    bf16
                #   t2   = ln(W0 - 1)                   f32   [0,4RT)
                #   W1   = exp(1.2*t2 + ln2) = d2 + 2   bf16  [0,4RT)
                lt = pool.tile([P, w6], F32, tag="lt")
                nc.scalar.activation(lt[:], s[:], AF.Ln)
                dist = pool.tile([P, w6], BF16, tag="dist")
                nc.scalar.activation(dist[:], lt[:], AF.Exp, scale=0.5)
                nc.scalar.activation(lt[:], dist[:], AF.Ln, bias=1.0)
                W0 = pool.tile([P, w6], BF16, tag="W0")
                nc.scalar.activation(W0[:], lt[:], AF.Exp, scale=1.2, bias=LN2)
                t2 = lt[:, 0:w4]
                nc.scalar.activation(t2, W0[:, 0:w4], AF.Ln, bias=-1.0)
                W1 = s[:, 0:w4]  # s dead after ln(s)
                nc.scalar.activation(W1, t2, AF.Exp, scale=1.2, bias=LN2)

                # select-by-max: res = max(dist, n*(W0-2), m2*(W1-2));
                # u1 reuses sqx, res/u2 reuse delta (all DVE-serial)
                u1 = sqx[:]
                nc.vector.scalar_tensor_tensor(
                    u1, W0[:], 2.0, n[:], ALU.subtract, ALU.mult)
                nc.vector.tensor_tensor(delta[:, 0:w6], dist[:], u1, ALU.max)
                u2 = delta[:, w6:w6 + w4]
                nc.vector.scalar_tensor_tensor(
                    u2, W1, 2.0, m2[:], ALU.subtract, ALU.mult)
                nc.vector.tensor_tensor(delta[:, 0:w4], delta[:, 0:w4], u2,
                                        ALU.max)

                # row sums: contiguous add tree over res=delta[:,0:w6]
                # (pair order irrelevant); final level writes f32 output
                nc.vector.tensor_tensor(delta[:, 0:w3], delta[:, 0:w3],
                                        delta[:, w3:2 * w3], ALU.add)
                nc.vector.tensor_tensor(delta[:, 0:rt], delta[:, 0:rt],
                                        delta[:, rt:2 * rt], ALU.add)
                ot = pool.tile([P, rt], F32, tag="ot")
                nc.vector.tensor_tensor(ot[:], delta[:, 0:rt],
                                        delta[:, 2 * rt:w3], ALU.add)
                nc.sync.dma_start(out=o[i], in_=ot[:])
    nc.compile()
    return nc


_NC_CACHE: dict = {}


def _get_nc(rt: int = RT, nt: int = NT):
    key = (rt, nt)
    if key not in _NC_CACHE:
        _NC_CACHE[key] = build_nc(rt, nt)
    return _NC_CACHE[key]


# ---------------------------------------------------------------- host shard


def make_in_maps(a: np.ndarray, b: np.ndarray, rt: int = RT, nt: int = NT):
    """Shard + component-major permute: [B,12] -> 8 x [nt, P, 12*rt]."""
    rows_pc = P * rt * nt
    perm = np.asarray(COMP_PERM, dtype=np.int64)

    def shard(x):
        sh = np.zeros((N_CORES, rows_pc, COLS), dtype=np.float32)
        sh[:, :ROWS_VALID, :] = x.reshape(N_CORES, ROWS_VALID, COLS)[..., perm]
        # [C, nt, P, rt, 12] -> [C, nt, P, 12, rt] -> [C, nt, P, 12*rt]
        sh = sh.reshape(N_CORES, nt, P, rt, COLS)
        sh = np.ascontiguousarray(sh.transpose(0, 1, 2, 4, 3))
        return sh.reshape(N_CORES, nt, P, COLS * rt)

    a_sh = shard(a)
    b_sh = shard(b)
    return [
        {"output": a_sh[c], "target": b_sh[c]} for c in range(N_CORES)
    ]


# ---------------------------------------------------------------- entrypoint


def kernel(output, target):
    a = np.asarray(output, dtype=np.float32)
    b = np.asarray(target, dtype=np.float32)
    assert a.shape == (B, COLS) and b.shape == (B, COLS)

    nc = _get_nc()
    in_maps = make_in_maps(a, b)
    r = run_bass_kernel_spmd(nc, in_maps, list(range(N_CORES)))
    out = np.empty((N_CORES, ROWS_VALID), dtype=np.float32)
    for c in range(N_CORES):
        loss = r.results[c]["loss"].reshape(NT * P * RT)
        out[c] = loss[:ROWS_VALID]
    return out.reshape(B)
